# revision 4
# baseline (speedup 1.0000x reference)
"""Trainium2 Bass kernel for nn_LocalExperts (MoE expert-parallel FFN).

Reference computation (per full input):
    x  [T=16384, D=1024] -> reshape [E=8, C=2048, D]
    h  = gelu(x @ w1[e] + b1[e])     w1 [E, D, F=4096]
    y  = h @ w2[e] + b2[e]           w2 [E, F, D]
    out[T, D]

Sharding: expert parallelism across 8 NeuronCores. Expert e's tokens are
exactly rows [e*C:(e+1)*C] of the input, so core e gets that token slice
plus w1[e], b1[e], w2[e], b2[e]. No collectives; outputs are concatenated
on the host.

Host-side marshalling (part of the sharding layout, no FLOPs):
  - X slice is transposed to Xt [D, C] so the contraction dim lands on
    SBUF partitions via plain DMA (kills all PE transposes + staging).
  - X/w1/w2 are cast to bf16 (PE runs 1 cycle/row for bf16 same as
    fp32r, but DMA/SBUF halve and weight loads use FWL). rel-err ~3e-3,
    well inside the 2e-2 gate. b1 is pre-arranged to [128, F/128] so the
    per-f-tile ACT bias column is a contiguous 128B DMA line.

Per-core kernel (C=2048 tokens, one expert), single pass over tokens:
  - Xt [128(d), 8, 2048] bf16 resident; yacc [128(c), 16, 1024] fp32
    accumulator resident.
  - Loop F in chunks of FC=512 (w1/w2 chunk tiles double-buffered,
    streamed once - 16.8 MB total):
      GEMM1: Ht[f,c] = gelu(W1c.T @ Xt + b1)  (PSUM acc over 8 d-tiles,
                                               ACT gelu drains to bf16)
      GEMM2: Yacc[c,d] += Ht.T @ W2c          (PSUM acc over 4 f-tiles,
                                               DVE acc over 8 chunks)
  - Yacc initialized with broadcast b2 on chunk 0; per [128,512] tile
    writeback on the last chunk.
"""

import os
from contextlib import ExitStack

import numpy as np
import ml_dtypes

import concourse.bass as bass
import concourse.tile as tile
from concourse import bacc
from concourse import mybir
from concourse.bass import ds, ts
from concourse.bass_utils import run_bass_kernel_spmd
from concourse.masks import make_identity

AFT = mybir.ActivationFunctionType

E = 8
D = 1024
F = 4096
T = 16384
C = T // E          # tokens per core
P = 128

FC = 512            # F chunk per iteration
NFREE = 512         # matmul moving free dim (one PSUM bank of fp32)
D_T = D // P        # 8 d-tiles
FC_T = FC // P      # 4 f-tiles per chunk
N_FC = F // FC      # 8 chunks
C_T = C // P        # 16 token tiles
N_CC = C // NFREE   # 4 token chunks

MM_MODE = "bf16"    # informational (test.py prints it)
# test-only: CoreSim lacks Gelu; "tanh" swaps the activation for sim gating
ACT_FN = os.environ.get("KERNEL_ACT", "gelu")


def _emit(ctx: ExitStack, tc: tile.TileContext, xt, w1, b1t, w2, b2, y):
    nc = tc.nc
    f32 = mybir.dt.float32
    bf16 = mybir.dt.bfloat16

    consts = ctx.enter_context(tc.tile_pool(name="consts", bufs=1))
    xt_pool = ctx.enter_context(tc.tile_pool(name="xt", bufs=1))
    yacc_pool = ctx.enter_context(tc.tile_pool(name="yacc", bufs=1))
    w1_pool = ctx.enter_context(tc.tile_pool(name="w1c", bufs=2))
    w2_pool = ctx.enter_context(tc.tile_pool(name="w2c", bufs=2))
    ht_pool = ctx.enter_context(tc.tile_pool(name="ht", bufs=2))
    mm_psum = ctx.enter_context(tc.tile_pool(name="mmp", bufs=8, space="PSUM"))

    # b1t arrives host-pre-arranged as [128, F_T]: column ft = per-partition
    # bias of f-tile ft; contiguous 128B per partition.
    F_T = F // P
    b1t_sb = consts.tile([P, F_T], f32)
    nc.sync.dma_start(b1t_sb[:], b1t)

    # Warm the PE HAM clock (cold 1.2GHz -> 2.4GHz needs ~3.4us of activity)
    # during the initial DMA wait, using identity matmuls.
    identity = consts.tile([P, P], f32)
    make_identity(nc, identity[:])
    warm_ps = mm_psum.tile([P, NFREE], f32, tag="mm")
    for _ in range(12):
        nc.tensor.matmul(warm_ps[:, :P], lhsT=identity[:], rhs=identity[:],
                         start=True, stop=True)

    w1_r = w1.rearrange("(do p) f -> p do f", p=P)    # [128, 8, 4096]
    w2_r = w2.rearrange("(fo p) d -> p fo d", p=P)    # [128, 32, 1024]
    xt_r = xt.rearrange("(dt p) c -> p dt c", p=P)    # [128, 8, 2048]

    # Startup DMAs split across both hwdge queues so the first GEMM1 group's
    # operands (w1c0 + xt chunk 0, ~2MB) stream in parallel (~3.5us):
    #   sync:   b1t, w1c0, xt_c2, w2c0, xt_c3, [weight prefetch loop...]
    #   scalar: xt_c0, xt_c1, b2b, [y writebacks at the end]
    xt_sb = xt_pool.tile([P, D_T, C], bf16, tag="xt")
    nc.scalar.dma_start(xt_sb[:, :, ds(0, NFREE)], xt_r[:, :, ds(0, NFREE)])
    w1c0 = w1_pool.tile([P, D_T, FC], bf16, tag="w1c")
    nc.sync.dma_start(w1c0[:], w1_r[:, :, ds(0, FC)])
    nc.scalar.dma_start(xt_sb[:, :, ds(NFREE, NFREE)], xt_r[:, :, ds(NFREE, NFREE)])
    nc.sync.dma_start(
        xt_sb[:, :, ds(2 * NFREE, NFREE)], xt_r[:, :, ds(2 * NFREE, NFREE)]
    )
    w2c0 = w2_pool.tile([P, FC_T, D], bf16, tag="w2c")
    nc.sync.dma_start(w2c0[:], w2_r[:, ds(0, FC_T), :])
    nc.sync.dma_start(
        xt_sb[:, :, ds(3 * NFREE, NFREE)], xt_r[:, :, ds(3 * NFREE, NFREE)]
    )
    # b2 broadcast across partitions for the Yacc init; needed only by the
    # first GEMM2 drain ~30us in
    b2b = consts.tile([P, D], f32)
    nc.scalar.dma_start(b2b[:], b2[None, :].to_broadcast((P, D)))

    yacc = yacc_pool.tile([P, C_T, D], f32, tag="yacc")

    w1c, w2c = w1c0, w2c0
    for fci in range(N_FC):
        # prefetch next chunk's weights (double-buffered)
        if fci + 1 < N_FC:
            w1n = w1_pool.tile([P, D_T, FC], bf16, tag="w1c")
            nc.sync.dma_start(w1n[:], w1_r[:, :, ds((fci + 1) * FC, FC)])
            w2n = w2_pool.tile([P, FC_T, D], bf16, tag="w2c")
            nc.sync.dma_start(w2n[:], w2_r[:, ds((fci + 1) * FC_T, FC_T), :])

        # ---- GEMM1: Ht[f, c] = gelu(sum_d W1[d, f]^T Xt[d, c] + b1[f]) ----
        ht = ht_pool.tile([P, FC_T, C], bf16, tag="ht")
        for cci in range(N_CC):
            for fti in range(FC_T):
                ft_g = fci * FC_T + fti
                ps = mm_psum.tile([P, NFREE], f32, tag="mm")
                for di in range(D_T):
                    nc.tensor.matmul(
                        ps[:],
                        lhsT=w1c[:, di, ds(fti * P, P)],
                        rhs=xt_sb[:, di, ds(cci * NFREE, NFREE)],
                        start=(di == 0),
                        stop=(di == D_T - 1),
                    )
                nc.scalar.activation(
                    ht[:, fti, ds(cci * NFREE, NFREE)],
                    ps[:],
                    AFT.Tanh if ACT_FN == "tanh" else AFT.Gelu_apprx_tanh,
                    bias=b1t_sb[:, ft_g : ft_g + 1],
                    scale=1.0,
                )

        # ---- GEMM2: Yacc[c, d] += sum_f Ht[f, c]^T W2[f, d] ----
        for ci in range(C_T):
            for dci in range(D // NFREE):
                ps = mm_psum.tile([P, NFREE], f32, tag="mm")
                for fti in range(FC_T):
                    nc.tensor.matmul(
                        ps[:],
                        lhsT=ht[:, fti, ds(ci * P, P)],
                        rhs=w2c[:, fti, ds(dci * NFREE, NFREE)],
                        start=(fti == 0),
                        stop=(fti == FC_T - 1),
                    )
                ya = yacc[:, ci, ds(dci * NFREE, NFREE)]
                if fci == 0:
                    nc.vector.tensor_add(
                        out=ya, in0=ps[:], in1=b2b[:, ds(dci * NFREE, NFREE)]
                    )
                else:
                    nc.vector.tensor_add(out=ya, in0=ya, in1=ps[:])
                if fci == N_FC - 1:
                    # tile complete: writeback alternating across both hwdge
                    # queues (both idle by now) so the 8MB output drains at
                    # full DMA rate and the post-matmul flush stays ~1.5us
                    q = nc.scalar if (ci * 2 + dci) % 2 == 0 else nc.sync
                    q.dma_start(y[ds(ci * P, P), ds(dci * NFREE, NFREE)], ya)
        if fci + 1 < N_FC:
            w1c, w2c = w1n, w2n


_NC_CACHE = None


def build_bass():
    global _NC_CACHE
    if _NC_CACHE is not None:
        return _NC_CACHE
    nc = bacc.Bacc("TRN2", target_bir_lowering=False, debug=False)
    f32 = mybir.dt.float32
    bf16 = mybir.dt.bfloat16
    xt = nc.dram_tensor("xt", [D, C], bf16, kind="ExternalInput").ap()
    w1 = nc.dram_tensor("w1", [D, F], bf16, kind="ExternalInput").ap()
    b1t = nc.dram_tensor("b1t", [P, F // P], f32, kind="ExternalInput").ap()
    w2 = nc.dram_tensor("w2", [F, D], bf16, kind="ExternalInput").ap()
    b2 = nc.dram_tensor("b2", [D], f32, kind="ExternalInput").ap()
    y = nc.dram_tensor("y", [C, D], f32, kind="ExternalOutput").ap()
    with tile.TileContext(nc) as tc:
        with ExitStack() as ctx:
            _emit(ctx, tc, xt, w1, b1t, w2, b2, y)
    nc.compile()
    _NC_CACHE = nc
    return nc


def _in_maps(inputs, w1, b1, w2, b2):
    bf = ml_dtypes.bfloat16
    maps = []
    for e in range(E):
        xs = inputs[e * C : (e + 1) * C]
        maps.append(
            {
                "xt": np.ascontiguousarray(xs.T).astype(bf),
                "w1": w1[e].astype(bf),
                "b1t": np.ascontiguousarray(
                    b1[e].astype(np.float32).reshape(F // P, P).T
                ),
                "w2": w2[e].astype(bf),
                "b2": np.ascontiguousarray(b2[e], dtype=np.float32),
            }
        )
    return maps


def kernel_run(inputs, w1, b1, w2, b2, trace=False, **trace_kwargs):
    """Run on 8 NeuronCores; returns (full_output [T, D], BassKernelResults)."""
    inputs = np.asarray(inputs, dtype=np.float32)
    w1 = np.asarray(w1, dtype=np.float32)
    b1 = np.asarray(b1, dtype=np.float32)
    w2 = np.asarray(w2, dtype=np.float32)
    b2 = np.asarray(b2, dtype=np.float32)
    nc = build_bass()
    res = run_bass_kernel_spmd(
        nc,
        _in_maps(inputs, w1, b1, w2, b2),
        core_ids=list(range(E)),
        trace=trace,
        **trace_kwargs,
    )
    out = np.concatenate([res.results[e]["y"] for e in range(E)], axis=0)
    return out, res


def kernel(inputs, w1, b1, w2, b2):
    out, _ = kernel_run(inputs, w1, b1, w2, b2, trace=False)
    return out
